# revision 23
# baseline (speedup 1.0000x reference)
"""Trainium2 Bass kernel for nn_MultiHeadAttention_79224966742350.

Full (unsharded) inputs in, full output out. Internally: 8-way SPMD over
8 NeuronCores, sharded batch x head-group: core c handles batch c//4 and
heads [4*(c%4), 4*(c%4)+4) (=256 of the 1024 projection dims). Each core
computes its partial x @ wo_cols contribution; the host sums the 4
partials per batch and adds bo.

v2 design notes (evolution of v1; v0 baseline was 360us, v1 194us):
  * All matmul operands bf16 (fp32 PSUM accumulation); inputs are
    pre-transposed AND pre-cast on the host, so the device does zero
    layout preparation (no PE transposes, no transpose evictions).
  * Scores for the two heads of a 128-row pair go into one [128,2,512]
    PSUM tile; a single ACT Exp instruction covers both heads.
  * Projection (chunk j+1) and output-projection (chunk j-1) matmuls
    are woven between attention iterations of chunk j so the PE never
    idles on the score->exp->attnV dependency chain.
  * Prologue: weight/x DMAs ordered so the Q-projection's first
    matmul can start after ~2us (wq first, then xq chunk 0 split per
    contraction tile; subtile deps release each matmul individually).
  * Bias matmuls are compiled out when all biases are zero (the case
    here); otherwise Q/K biases ride the PSUM eviction (ACT Identity
    with a per-partition bias AP - same ACT table as Exp, no reload),
    and V keeps a K=1 ones matmul.
  * Output DMAs issue from GPSIMD (25ns sequencer cost vs 565ns on
    sync) to keep the sync queue free for input prefetch.
"""

import sys

sys.path.insert(0, "/opt/trn_rl_repo")

import numpy as np
import ml_dtypes

import concourse.bacc as bacc
import concourse.mybir as mybir
import concourse.tile as tile
from concourse.bass_utils import run_bass_kernel_spmd

F32 = mybir.dt.float32
BF16 = mybir.dt.bfloat16
AF = mybir.ActivationFunctionType
NPBF = ml_dtypes.bfloat16

B = 2
S = 2048
D = 1024
DK = 64
HPC = 4          # heads per core
HD = HPC * DK    # 256 projection dims per core
NCORES = 8
CH = 512         # q-chunk width (tokens)
NJ = S // CH     # 4 chunks
P = 128


def build_nc(with_bias):
    nc = bacc.Bacc("TRN2", target_bir_lowering=False, debug=False,
                   num_devices=NCORES)

    xqT = nc.dram_tensor("xqT", [D, S], BF16, kind="ExternalInput").ap()
    xkT = nc.dram_tensor("xkT", [D, S], BF16, kind="ExternalInput").ap()
    xvT = nc.dram_tensor("xvT", [D, S], BF16, kind="ExternalInput").ap()
    wqT = nc.dram_tensor("wqT", [D, HD], BF16, kind="ExternalInput").ap()
    wkT = nc.dram_tensor("wkT", [D, HD], BF16, kind="ExternalInput").ap()
    wvT = nc.dram_tensor("wvT", [D, HD], BF16, kind="ExternalInput").ap()
    woT = nc.dram_tensor("woT", [HD, D], BF16, kind="ExternalInput").ap()
    if with_bias:
        # bqc/bkc: [128, 2] per-partition bias columns (pair-tiled dh)
        bqc = nc.dram_tensor("bqc", [P, 2], F32, kind="ExternalInput").ap()
        bkc = nc.dram_tensor("bkc", [P, 2], F32, kind="ExternalInput").ap()
        bv = nc.dram_tensor("bv", [1, HD], BF16, kind="ExternalInput").ap()
    tri = nc.dram_tensor("tri", [P, P], BF16, kind="ExternalInput").ap()
    ones = nc.dram_tensor("ones", [1, CH], BF16, kind="ExternalInput").ap()
    out = nc.dram_tensor("out", [S, D], F32, kind="ExternalOutput").ap()

    x_aps = {"q": xqT, "k": xkT, "v": xvT}

    with tile.TileContext(nc) as tc:
        with (
            tc.tile_pool(name="const", bufs=1) as const,
            tc.tile_pool(name="wtp", bufs=1) as wtp,
            tc.tile_pool(name="qkv", bufs=1) as qkv,
            tc.tile_pool(name="xtp", bufs=2) as xtp,
            tc.tile_pool(name="qc", bufs=2) as qcp,
            tc.tile_pool(name="opool", bufs=3) as opool,
            tc.tile_pool(name="pp", bufs=3) as pp,
            tc.tile_pool(name="small", bufs=2) as small,
            tc.tile_pool(name="outsb", bufs=3) as outsbp,
            tc.tile_pool(name="psS", bufs=2, space="PSUM") as psS,
            tc.tile_pool(name="psO", bufs=2, space="PSUM") as psO,
            tc.tile_pool(name="psPW", bufs=2, space="PSUM") as psPW,
        ):
            # ---- weight/const tiles ----------------------------------
            wq_sb = wtp.tile([P, 8, HD], BF16, tag="wq")
            wk_sb = wtp.tile([P, 8, HD], BF16, tag="wk")
            wv_sb = wtp.tile([P, 8, HD], BF16, tag="wv")
            wo_sb = wtp.tile([P, 2, D], BF16, tag="wo")
            tri_sb = const.tile([P, P], BF16, tag="tri")
            ones_sb = const.tile([1, CH], BF16, tag="ones")
            if with_bias:
                bqc_sb = wtp.tile([P, 2], F32, tag="bqc")
                bkc_sb = wtp.tile([P, 2], F32, tag="bkc")
                bv_sb = wtp.tile([1, HD], BF16, tag="bv")

            xts = {}       # (name, j) -> staged x^T tile
            q_tiles = {}   # j -> [Qc0, Qc1]
            o_tiles = {}   # j -> [O0, O1]

            def dma_x(j, split):
                """Stage x^T chunk j. split=True: one DMA per 128-row
                contraction tile (releases matmuls early via subtile
                deps); else one DMA per input."""
                for name in ("q", "k", "v"):
                    ap = x_aps[name].rearrange("(d p) t -> p d t", p=P)
                    xt = xtp.tile([P, 8, CH], BF16, tag=f"x{name}",
                                  name=f"x{name}{j}")
                    xts[(name, j)] = xt
                    if split:
                        for d in range(8):
                            nc.sync.dma_start(
                                out=xt[:, d, :],
                                in_=ap[:, d, j * CH:(j + 1) * CH])
                    else:
                        nc.sync.dma_start(
                            out=xt[:], in_=ap[:, :, j * CH:(j + 1) * CH])

            def prologue_dma():
                # ordered so the first Q-proj matmuls unblock earliest
                for name, wt, wsb in (("q", wqT, wq_sb), ("k", wkT, wk_sb),
                                      ("v", wvT, wv_sb)):
                    wap = wt.rearrange("(d p) h -> p d h", p=P)
                    xap = x_aps[name].rearrange("(d p) t -> p d t", p=P)
                    xt = xtp.tile([P, 8, CH], BF16, tag=f"x{name}",
                                  name=f"x{name}0")
                    xts[(name, 0)] = xt
                    nc.sync.dma_start(out=wsb[:], in_=wap)
                    for d in range(8):
                        nc.sync.dma_start(out=xt[:, d, :],
                                          in_=xap[:, d, 0:CH])
                nc.sync.dma_start(out=wo_sb[:],
                                  in_=woT.rearrange("(m p) n -> p m n", p=P))
                nc.sync.dma_start(out=tri_sb[:], in_=tri)
                nc.sync.dma_start(out=ones_sb[:], in_=ones)
                if with_bias:
                    nc.sync.dma_start(out=bqc_sb[:], in_=bqc)
                    nc.sync.dma_start(out=bkc_sb[:], in_=bkc)
                    nc.sync.dma_start(out=bv_sb[:], in_=bv)

            def proj_steps(j):
                """Generator: one PE matmul or eviction per yield."""
                q_tiles[j] = [qcp.tile([P, CH], BF16, tag=f"q{m}",
                                       name=f"Qc{m}_{j}")
                              for m in range(2)]
                for name, wsb in (("q", wq_sb), ("k", wk_sb)):
                    xt = xts[(name, j)]
                    for m in range(2):
                        ps = psPW.tile([P, CH], F32, tag="pw", name="psp")
                        for d in range(8):
                            nc.tensor.matmul(
                                ps[:], wsb[:, d, m * P:(m + 1) * P],
                                xt[:, d, :], start=(d == 0), stop=(d == 7))
                            yield
                        dst = (q_tiles[j][m][:] if name == "q"
                               else K_sb[m][:, j * CH:(j + 1) * CH])
                        if with_bias:
                            # DVE mult+add would cost more; ACT Identity
                            # shares the Exp table so no table reload.
                            bc = bqc_sb if name == "q" else bkc_sb
                            nc.scalar.activation(dst, ps[:], AF.Identity,
                                                 bias=bc[:, m:m + 1])
                        else:
                            # keep ACT exp-only: evict on DVE
                            nc.vector.tensor_copy(dst, ps[:])
                        yield
                xt = xts[("v", j)]
                for t in range(4):
                    ps = psPW.tile([P, CH], F32, tag="pw", name="psv")
                    for d in range(8):
                        nc.tensor.matmul(
                            ps[:, 0:HD], xt[:, d, t * P:(t + 1) * P],
                            wv_sb[:, d, :], start=(d == 0),
                            stop=(d == 7 and not with_bias))
                        yield
                    if with_bias:
                        nc.tensor.matmul(
                            ps[:, 0:HD], ones_sb[0:1, 0:P], bv_sb[:],
                            start=False, stop=True)
                        yield
                    nc.vector.tensor_copy(
                        V_sb[:, j * 4 + t, :, 0:DK],
                        ps[:, 0:HD].rearrange("p (h c) -> p h c", c=DK))
                    yield

            def wo_steps(j, act_ok=False):
                """Generator: output projection for chunk j. act_ok: the
                ACT engine has no more exps at this point, so osb
                evictions may alternate onto it."""
                O0, O1 = o_tiles[j]
                for t in range(4):
                    for n in range(2):
                        ps = psPW.tile([P, CH], F32, tag="pw", name="psw")
                        nc.tensor.matmul(
                            ps[:], O0[:, t * P:(t + 1) * P],
                            wo_sb[:, 0, n * CH:(n + 1) * CH],
                            start=True, stop=False)
                        yield
                        nc.tensor.matmul(
                            ps[:], O1[:, t * P:(t + 1) * P],
                            wo_sb[:, 1, n * CH:(n + 1) * CH],
                            start=False, stop=True)
                        yield
                        osb = outsbp.tile([P, CH], F32, tag="osb")
                        if act_ok and (t + n) % 2 == 0:
                            nc.scalar.activation(osb[:], ps[:], AF.Copy)
                        else:
                            nc.vector.tensor_copy(osb[:], ps[:])
                        yield
                        rows = slice(j * CH + t * P, j * CH + (t + 1) * P)
                        if j == NJ - 1:
                            # final chunk: quarter the DMA across queues
                            # so the drain tail is ~64KB deep, not 256KB
                            for q in range(4):
                                cs = n * CH + q * (CH // 4)
                                nc.sync.dma_start(
                                    out=out[rows, cs:cs + CH // 4],
                                    in_=osb[:, q * (CH // 4):
                                            (q + 1) * (CH // 4)])
                        else:
                            nc.sync.dma_start(
                                out=out[rows, n * CH:(n + 1) * CH],
                                in_=osb[:])
                        yield

            def do_att(j, weave_steps, total, reserve=0):
                """Attention for q-chunk j, weaving the given generators
                into the PE stream between iterations. `total` is the
                step count across the generators; the last `reserve`
                steps are held back for the post-loop drain (they fill
                the PE while the final normalize chain completes)."""
                weave = [iter(g) for g in weave_steps]
                consumed = 0
                loop_budget = total - reserve

                def pump(n, drain=False):
                    nonlocal consumed
                    for _ in range(n):
                        if not drain and consumed >= loop_budget:
                            return
                        while weave:
                            try:
                                next(weave[0])
                                consumed += 1
                                break
                            except StopIteration:
                                weave.pop(0)
                        if not weave:
                            return

                o_tiles[j] = [opool.tile([P, CH], BF16, tag=f"o{m}",
                                         name=f"O{m}_{j}")
                              for m in range(2)]
                nk = 4 * (j + 1)
                quota = max(1, -(-total // (2 * nk)))  # ~even distribution

                with nc.named_scope(f"att{j}"):
                    for pair in range(2):
                        m = pair
                        hA, hB = 2 * pair, 2 * pair + 1
                        Qc = q_tiles[j][m]
                        poA = psO.tile([DK + 1, CH], F32, tag="po",
                                       name="poA")
                        poB = psO.tile([DK + 1, CH], F32, tag="po",
                                       name="poB")

                        def c0(i):
                            return max(0, P * i - CH * j)

                        def score(i):
                            cc = c0(i)
                            s = psS.tile([P, 2, CH], F32, tag="s", name="s")
                            p = pp.tile([P, 2, CH], BF16, tag="p", name="p")
                            nc.tensor.matmul(
                                s[:, 0, cc:CH],
                                K_sb[m][0:DK, i * P:(i + 1) * P],
                                Qc[0:DK, cc:CH], start=True, stop=True)
                            nc.tensor.matmul(
                                s[:, 1, cc:CH],
                                K_sb[m][DK:P, i * P:(i + 1) * P],
                                Qc[DK:P, cc:CH], start=True, stop=True)
                            nc.scalar.activation(
                                p[:, :, cc:CH], s[:, :, cc:CH], AF.Exp,
                                scale=0.125)
                            if i >= 4 * j:
                                nc.vector.tensor_mul(
                                    p[:, 0, cc:cc + P], p[:, 0, cc:cc + P],
                                    tri_sb[:])
                                nc.vector.tensor_mul(
                                    p[:, 1, cc:cc + P], p[:, 1, cc:cc + P],
                                    tri_sb[:])
                            return p

                        prev = score(0)
                        for i in range(nk):
                            p = prev
                            if i + 1 < nk:
                                prev = score(i + 1)
                            cc = c0(i)
                            nc.tensor.matmul(
                                poA[:, cc:CH], V_sb[:, i, hA, :],
                                p[:, 0, cc:CH], start=(i == 0),
                                stop=(i == nk - 1))
                            nc.tensor.matmul(
                                poB[:, cc:CH], V_sb[:, i, hB, :],
                                p[:, 1, cc:CH], start=(i == 0),
                                stop=(i == nk - 1))
                            pump(quota)

                        for off, po in ((0, poA), (DK, poB)):
                            dsb = small.tile([1, CH], F32, tag="dsb")
                            nc.vector.tensor_copy(dsb[:], po[DK:DK + 1, :])
                            r = small.tile([1, CH], F32, tag="r")
                            nc.vector.reciprocal_approx_fast(r[:], dsb[:])
                            rb = small.tile([DK, CH], F32, tag="rb")
                            nc.gpsimd.partition_broadcast(rb[:], r[:],
                                                          channels=DK)
                            # per column-half so downstream wo token
                            # tiles unblock progressively
                            for h0 in (0, CH // 2):
                                nc.vector.tensor_mul(
                                    o_tiles[j][m][off:off + DK,
                                                  h0:h0 + CH // 2],
                                    po[0:DK, h0:h0 + CH // 2],
                                    rb[:, h0:h0 + CH // 2])
                    pump(1 << 30, drain=True)  # drain remaining weave

            # ---- persistent K^T (pair-tiled) and V (+ ones col) -------
            K_sb = [qkv.tile([P, S], BF16, tag=f"K{m}", name=f"K{m}")
                    for m in range(2)]
            V_sb = qkv.tile([P, S // P, HPC, DK + 1], BF16, tag="V")

            # ---- schedule ---------------------------------------------
            prologue_dma()
            nc.gpsimd.memset(V_sb[:, :, :, DK:DK + 1], 1.0)
            dma_x(1, split=False)
            with nc.named_scope("proj0"):
                for _ in proj_steps(0):
                    pass
            dma_x(2, split=False)
            do_att(0, [proj_steps(1)], total=72)
            dma_x(3, split=False)
            do_att(1, [wo_steps(0), proj_steps(2)], total=104)
            do_att(2, [proj_steps(3)], total=72)
            do_att(3, [wo_steps(1), wo_steps(2, act_ok=True)],
                   total=64, reserve=20)
            with nc.named_scope("wo3"):
                for _ in wo_steps(3, act_ok=True):
                    pass

    nc.compile()
    return nc


_NC_CACHE = {}
_last_in_maps = None


def _get_nc(with_bias=False):
    if with_bias not in _NC_CACHE:
        _NC_CACHE[with_bias] = build_nc(with_bias)
    return _NC_CACHE[with_bias]


def _reference_np(q, k, v, mask, wq, bq, wk, bk, wv, bv, wo, bo):
    """Plain numpy fallback (only used if mask is not causal)."""
    query = q @ wq.T + bq
    key_ = k @ wk.T + bk
    value = v @ wv.T + bv
    H = D // DK
    query = query.reshape(B, S, H, DK).transpose(0, 2, 1, 3)
    key_ = key_.reshape(B, S, H, DK).transpose(0, 2, 1, 3)
    value = value.reshape(B, S, H, DK).transpose(0, 2, 1, 3)
    scores = np.einsum("bhqd,bhkd->bhqk", query, key_) / np.sqrt(np.float32(DK))
    scores = np.where(mask == 0, np.float32(-1e9), scores)
    scores = scores - scores.max(axis=-1, keepdims=True)
    e = np.exp(scores)
    attn = e / e.sum(axis=-1, keepdims=True)
    x = np.einsum("bhqk,bhkd->bhqd", attn, value)
    x = x.transpose(0, 2, 1, 3).reshape(B, S, D)
    return (x @ wo.T + bo).astype(np.float32)


def kernel(q, k, v, mask, wq, bq, wk, bk, wv, bv, wo, bo, **_unused):
    q = np.asarray(q, np.float32)
    k = np.asarray(k, np.float32)
    v = np.asarray(v, np.float32)
    wq = np.asarray(wq, np.float32)
    wk = np.asarray(wk, np.float32)
    wv = np.asarray(wv, np.float32)
    wo = np.asarray(wo, np.float32)
    bq = np.asarray(bq, np.float32)
    bk = np.asarray(bk, np.float32)
    bv = np.asarray(bv, np.float32)
    bo = np.asarray(bo, np.float32)
    mask_np = np.asarray(mask)

    # the device kernel hardcodes causal masking; verify and fall back if not
    causal = np.tril(np.ones((S, S), np.int32))
    if not np.array_equal(mask_np.reshape(S, S).astype(np.int32), causal):
        return _reference_np(q, k, v, mask_np, wq, bq, wk, bk, wv, bv, wo, bo)

    with_bias = bool(np.any(bq) or np.any(bk) or np.any(bv))
    nc = _get_nc(with_bias)

    # host-side prep: transpose + cast to bf16 once per batch / core
    xT = {}
    for b in range(B):
        xT[b] = (q[b].T.astype(NPBF), k[b].T.astype(NPBF),
                 v[b].T.astype(NPBF))
    tri_np = np.triu(np.ones((P, P), NPBF))
    ones_np = np.ones((1, CH), NPBF)

    in_maps = []
    for c in range(NCORES):
        b, g = c // 4, c % 4
        sl = slice(g * HD, (g + 1) * HD)
        xq_b, xk_b, xv_b = xT[b]
        im = {
            "xqT": xq_b,
            "xkT": xk_b,
            "xvT": xv_b,
            "wqT": wq[sl].T.astype(NPBF),
            "wkT": wk[sl].T.astype(NPBF),
            "wvT": wv[sl].T.astype(NPBF),
            "woT": wo[:, sl].T.astype(NPBF),
            "tri": tri_np,
            "ones": ones_np,
        }
        if with_bias:
            im["bqc"] = np.ascontiguousarray(
                bq[sl].reshape(2, P).T.astype(np.float32))
            im["bkc"] = np.ascontiguousarray(
                bk[sl].reshape(2, P).T.astype(np.float32))
            im["bv"] = bv[sl].reshape(1, HD).astype(NPBF)
        in_maps.append(im)

    global _last_in_maps
    _last_in_maps = in_maps
    res = run_bass_kernel_spmd(nc, in_maps, core_ids=list(range(NCORES)))

    out = np.empty((B, S, D), np.float32)
    for b in range(B):
        acc = res.results[4 * b]["out"].astype(np.float32).copy()
        for g in range(1, 4):
            acc += res.results[4 * b + g]["out"]
        out[b] = acc + bo[None, :]
    return out


# revision 27
# speedup vs baseline: 1.0462x; 1.0462x over previous
"""Trainium2 Bass kernel for nn_MultiHeadAttention_79224966742350.

Full (unsharded) inputs in, full output out. Internally: 8-way SPMD over
8 NeuronCores, sharded batch x head-group: core c handles batch c//4 and
heads [4*(c%4), 4*(c%4)+4) (=256 of the 1024 projection dims). Each core
computes its partial x @ wo_cols contribution; the host sums the 4
partials per batch and adds bo.

v2 design notes (evolution of v1; v0 baseline was 360us, v1 194us):
  * All matmul operands bf16 (fp32 PSUM accumulation); inputs are
    pre-transposed AND pre-cast on the host, so the device does zero
    layout preparation (no PE transposes, no transpose evictions).
  * Scores for the two heads of a 128-row pair go into one [128,2,512]
    PSUM tile; a single ACT Exp instruction covers both heads.
  * Projection (chunk j+1) and output-projection (chunk j-1) matmuls
    are woven between attention iterations of chunk j so the PE never
    idles on the score->exp->attnV dependency chain.
  * Prologue: weight/x DMAs ordered so the Q-projection's first
    matmul can start after ~2us (wq first, then xq chunk 0 split per
    contraction tile; subtile deps release each matmul individually).
  * Bias matmuls are compiled out when all biases are zero (the case
    here); otherwise Q/K biases ride the PSUM eviction (ACT Identity
    with a per-partition bias AP - same ACT table as Exp, no reload),
    and V keeps a K=1 ones matmul.
  * Output DMAs issue from GPSIMD (25ns sequencer cost vs 565ns on
    sync) to keep the sync queue free for input prefetch.
"""

import sys

sys.path.insert(0, "/opt/trn_rl_repo")

import numpy as np
import ml_dtypes

import concourse.bacc as bacc
import concourse.mybir as mybir
import concourse.tile as tile
from concourse.bass_utils import run_bass_kernel_spmd

F32 = mybir.dt.float32
BF16 = mybir.dt.bfloat16
AF = mybir.ActivationFunctionType
NPBF = ml_dtypes.bfloat16

B = 2
S = 2048
D = 1024
DK = 64
HPC = 4          # heads per core
HD = HPC * DK    # 256 projection dims per core
NCORES = 8
CH = 512         # q-chunk width (tokens)
NJ = S // CH     # 4 chunks
P = 128


def build_nc(with_bias):
    nc = bacc.Bacc("TRN2", target_bir_lowering=False, debug=False,
                   num_devices=NCORES)

    xqT = nc.dram_tensor("xqT", [D, S], BF16, kind="ExternalInput").ap()
    xkT = nc.dram_tensor("xkT", [D, S], BF16, kind="ExternalInput").ap()
    xvT = nc.dram_tensor("xvT", [D, S], BF16, kind="ExternalInput").ap()
    wqT = nc.dram_tensor("wqT", [D, HD], BF16, kind="ExternalInput").ap()
    wkT = nc.dram_tensor("wkT", [D, HD], BF16, kind="ExternalInput").ap()
    wvT = nc.dram_tensor("wvT", [D, HD], BF16, kind="ExternalInput").ap()
    woT = nc.dram_tensor("woT", [HD, D], BF16, kind="ExternalInput").ap()
    if with_bias:
        # bqc/bkc: [128, 2] per-partition bias columns (pair-tiled dh)
        bqc = nc.dram_tensor("bqc", [P, 2], F32, kind="ExternalInput").ap()
        bkc = nc.dram_tensor("bkc", [P, 2], F32, kind="ExternalInput").ap()
        bv = nc.dram_tensor("bv", [1, HD], BF16, kind="ExternalInput").ap()
    tri = nc.dram_tensor("tri", [P, P], BF16, kind="ExternalInput").ap()
    ones = nc.dram_tensor("ones", [1, CH], BF16, kind="ExternalInput").ap()
    out = nc.dram_tensor("out", [S, D], F32, kind="ExternalOutput").ap()

    x_aps = {"q": xqT, "k": xkT, "v": xvT}

    with tile.TileContext(nc) as tc:
        with (
            tc.tile_pool(name="const", bufs=1) as const,
            tc.tile_pool(name="wtp", bufs=1) as wtp,
            tc.tile_pool(name="qkv", bufs=1) as qkv,
            tc.tile_pool(name="xtp", bufs=2) as xtp,
            tc.tile_pool(name="qc", bufs=2) as qcp,
            tc.tile_pool(name="opool", bufs=3) as opool,
            tc.tile_pool(name="pp", bufs=3) as pp,
            tc.tile_pool(name="small", bufs=2) as small,
            tc.tile_pool(name="outsb", bufs=3) as outsbp,
            tc.tile_pool(name="psS", bufs=2, space="PSUM") as psS,
            tc.tile_pool(name="psO", bufs=2, space="PSUM") as psO,
            tc.tile_pool(name="psPW", bufs=2, space="PSUM") as psPW,
        ):
            # ---- weight/const tiles ----------------------------------
            wq_sb = wtp.tile([P, 8, HD], BF16, tag="wq")
            wk_sb = wtp.tile([P, 8, HD], BF16, tag="wk")
            wv_sb = wtp.tile([P, 8, HD], BF16, tag="wv")
            wo_sb = wtp.tile([P, 2, D], BF16, tag="wo")
            tri_sb = const.tile([P, P], BF16, tag="tri")
            ones_sb = const.tile([1, CH], BF16, tag="ones")
            if with_bias:
                bqc_sb = wtp.tile([P, 2], F32, tag="bqc")
                bkc_sb = wtp.tile([P, 2], F32, tag="bkc")
                bv_sb = wtp.tile([1, HD], BF16, tag="bv")

            xts = {}       # (name, j) -> staged x^T tile
            q_tiles = {}   # j -> [Qc0, Qc1]
            o_tiles = {}   # j -> [O0, O1]

            def dma_x(j, split):
                """Stage x^T chunk j. split=True: one DMA per 128-row
                contraction tile (releases matmuls early via subtile
                deps); else one DMA per input."""
                for name in ("q", "k", "v"):
                    ap = x_aps[name].rearrange("(d p) t -> p d t", p=P)
                    xt = xtp.tile([P, 8, CH], BF16, tag=f"x{name}",
                                  name=f"x{name}{j}")
                    xts[(name, j)] = xt
                    if split:
                        for d in range(8):
                            nc.sync.dma_start(
                                out=xt[:, d, :],
                                in_=ap[:, d, j * CH:(j + 1) * CH])
                    else:
                        nc.sync.dma_start(
                            out=xt[:], in_=ap[:, :, j * CH:(j + 1) * CH])

            def prologue_dma():
                # ordered so the first Q-proj matmuls unblock earliest
                for name, wt, wsb in (("q", wqT, wq_sb), ("k", wkT, wk_sb),
                                      ("v", wvT, wv_sb)):
                    wap = wt.rearrange("(d p) h -> p d h", p=P)
                    xap = x_aps[name].rearrange("(d p) t -> p d t", p=P)
                    xt = xtp.tile([P, 8, CH], BF16, tag=f"x{name}",
                                  name=f"x{name}0")
                    xts[(name, 0)] = xt
                    nc.sync.dma_start(out=wsb[:], in_=wap)
                    for d in range(8):
                        nc.sync.dma_start(out=xt[:, d, :],
                                          in_=xap[:, d, 0:CH])
                nc.sync.dma_start(out=wo_sb[:],
                                  in_=woT.rearrange("(m p) n -> p m n", p=P))
                nc.sync.dma_start(out=tri_sb[:], in_=tri)
                nc.sync.dma_start(out=ones_sb[:], in_=ones)
                if with_bias:
                    nc.sync.dma_start(out=bqc_sb[:], in_=bqc)
                    nc.sync.dma_start(out=bkc_sb[:], in_=bkc)
                    nc.sync.dma_start(out=bv_sb[:], in_=bv)

            def proj_steps(j):
                """Generator: one PE matmul or eviction per yield."""
                q_tiles[j] = [qcp.tile([P, CH], BF16, tag=f"q{m}",
                                       name=f"Qc{m}_{j}")
                              for m in range(2)]
                for name, wsb in (("q", wq_sb), ("k", wk_sb)):
                    xt = xts[(name, j)]
                    for m in range(2):
                        ps = psPW.tile([P, CH], F32, tag="pw", name="psp")
                        for d in range(8):
                            nc.tensor.matmul(
                                ps[:], wsb[:, d, m * P:(m + 1) * P],
                                xt[:, d, :], start=(d == 0), stop=(d == 7))
                            yield
                        dst = (q_tiles[j][m][:] if name == "q"
                               else K_sb[m][:, j * CH:(j + 1) * CH])
                        if with_bias:
                            # DVE mult+add would cost more; ACT Identity
                            # shares the Exp table so no table reload.
                            bc = bqc_sb if name == "q" else bkc_sb
                            nc.scalar.activation(dst, ps[:], AF.Identity,
                                                 bias=bc[:, m:m + 1])
                        else:
                            # keep ACT exp-only: evict on DVE
                            nc.vector.tensor_copy(dst, ps[:])
                        yield
                xt = xts[("v", j)]
                for t in range(4):
                    ps = psPW.tile([P, CH], F32, tag="pw", name="psv")
                    for d in range(8):
                        nc.tensor.matmul(
                            ps[:, 0:HD], xt[:, d, t * P:(t + 1) * P],
                            wv_sb[:, d, :], start=(d == 0),
                            stop=(d == 7 and not with_bias))
                        yield
                    if with_bias:
                        nc.tensor.matmul(
                            ps[:, 0:HD], ones_sb[0:1, 0:P], bv_sb[:],
                            start=False, stop=True)
                        yield
                    nc.vector.tensor_copy(
                        V_sb[:, j * 4 + t, :, 0:DK],
                        ps[:, 0:HD].rearrange("p (h c) -> p h c", c=DK))
                    yield

            def wo_steps(j, act_ok=False):
                """Generator: output projection for chunk j. act_ok: the
                ACT engine has no more exps at this point, so osb
                evictions may alternate onto it."""
                O0, O1 = o_tiles[j]
                for t in range(4):
                    for n in range(2):
                        ps = psPW.tile([P, CH], F32, tag="pw", name="psw")
                        nc.tensor.matmul(
                            ps[:], O0[:, t * P:(t + 1) * P],
                            wo_sb[:, 0, n * CH:(n + 1) * CH],
                            start=True, stop=False)
                        yield
                        nc.tensor.matmul(
                            ps[:], O1[:, t * P:(t + 1) * P],
                            wo_sb[:, 1, n * CH:(n + 1) * CH],
                            start=False, stop=True)
                        yield
                        osb = outsbp.tile([P, CH], F32, tag="osb")
                        if act_ok and (t + n) % 2 == 0:
                            nc.scalar.activation(osb[:], ps[:], AF.Copy)
                        else:
                            nc.vector.tensor_copy(osb[:], ps[:])
                        yield
                        nc.sync.dma_start(
                            out=out[j * CH + t * P:j * CH + (t + 1) * P,
                                    n * CH:(n + 1) * CH],
                            in_=osb[:])
                        yield

            def do_att(j, weave_steps):
                """Attention for q-chunk j, weaving the given generators
                into the PE stream between iterations."""
                weave = [iter(g) for g in weave_steps]

                def pump(n):
                    for _ in range(n):
                        while weave:
                            try:
                                next(weave[0])
                                break
                            except StopIteration:
                                weave.pop(0)
                        if not weave:
                            return

                o_tiles[j] = [opool.tile([P, CH], BF16, tag=f"o{m}",
                                         name=f"O{m}_{j}")
                              for m in range(2)]
                nk = 4 * (j + 1)
                quota = max(1, -(-120 // (2 * nk)))  # ~even distribution

                with nc.named_scope(f"att{j}"):
                    for pair in range(2):
                        m = pair
                        hA, hB = 2 * pair, 2 * pair + 1
                        Qc = q_tiles[j][m]
                        poA = psO.tile([DK + 1, CH], F32, tag="po",
                                       name="poA")
                        poB = psO.tile([DK + 1, CH], F32, tag="po",
                                       name="poB")

                        def c0(i):
                            return max(0, P * i - CH * j)

                        def score(i):
                            cc = c0(i)
                            s = psS.tile([P, 2, CH], F32, tag="s", name="s")
                            p = pp.tile([P, 2, CH], BF16, tag="p", name="p")
                            nc.tensor.matmul(
                                s[:, 0, cc:CH],
                                K_sb[m][0:DK, i * P:(i + 1) * P],
                                Qc[0:DK, cc:CH], start=True, stop=True)
                            nc.tensor.matmul(
                                s[:, 1, cc:CH],
                                K_sb[m][DK:P, i * P:(i + 1) * P],
                                Qc[DK:P, cc:CH], start=True, stop=True)
                            nc.scalar.activation(
                                p[:, :, cc:CH], s[:, :, cc:CH], AF.Exp,
                                scale=0.125)
                            if i >= 4 * j:
                                nc.vector.tensor_mul(
                                    p[:, 0, cc:cc + P], p[:, 0, cc:cc + P],
                                    tri_sb[:])
                                nc.vector.tensor_mul(
                                    p[:, 1, cc:cc + P], p[:, 1, cc:cc + P],
                                    tri_sb[:])
                            return p

                        prev = score(0)
                        for i in range(nk):
                            p = prev
                            if i + 1 < nk:
                                prev = score(i + 1)
                            cc = c0(i)
                            nc.tensor.matmul(
                                poA[:, cc:CH], V_sb[:, i, hA, :],
                                p[:, 0, cc:CH], start=(i == 0),
                                stop=(i == nk - 1))
                            nc.tensor.matmul(
                                poB[:, cc:CH], V_sb[:, i, hB, :],
                                p[:, 1, cc:CH], start=(i == 0),
                                stop=(i == nk - 1))
                            pump(quota)

                        for off, po in ((0, poA), (DK, poB)):
                            dsb = small.tile([1, CH], F32, tag="dsb")
                            nc.vector.tensor_copy(dsb[:], po[DK:DK + 1, :])
                            r = small.tile([1, CH], F32, tag="r")
                            nc.vector.reciprocal_approx_fast(r[:], dsb[:])
                            rb = small.tile([DK, CH], F32, tag="rb")
                            nc.gpsimd.partition_broadcast(rb[:], r[:],
                                                          channels=DK)
                            nc.vector.tensor_mul(
                                o_tiles[j][m][off:off + DK, :],
                                po[0:DK, :], rb[:])
                    pump(1 << 30)  # drain remaining weave steps

            # ---- persistent K^T (pair-tiled) and V (+ ones col) -------
            K_sb = [qkv.tile([P, S], BF16, tag=f"K{m}", name=f"K{m}")
                    for m in range(2)]
            V_sb = qkv.tile([P, S // P, HPC, DK + 1], BF16, tag="V")

            # ---- schedule ---------------------------------------------
            prologue_dma()
            nc.gpsimd.memset(V_sb[:, :, :, DK:DK + 1], 1.0)
            dma_x(1, split=False)
            with nc.named_scope("proj0"):
                for _ in proj_steps(0):
                    pass
            dma_x(2, split=False)
            do_att(0, [proj_steps(1)])
            dma_x(3, split=False)
            do_att(1, [wo_steps(0), proj_steps(2)])
            do_att(2, [proj_steps(3)])
            do_att(3, [wo_steps(1), wo_steps(2)])
            with nc.named_scope("wo3"):
                for _ in wo_steps(3, act_ok=True):
                    pass

    nc.compile()
    return nc


_NC_CACHE = {}
_last_in_maps = None


def _get_nc(with_bias=False):
    if with_bias not in _NC_CACHE:
        _NC_CACHE[with_bias] = build_nc(with_bias)
    return _NC_CACHE[with_bias]


def _reference_np(q, k, v, mask, wq, bq, wk, bk, wv, bv, wo, bo):
    """Plain numpy fallback (only used if mask is not causal)."""
    query = q @ wq.T + bq
    key_ = k @ wk.T + bk
    value = v @ wv.T + bv
    H = D // DK
    query = query.reshape(B, S, H, DK).transpose(0, 2, 1, 3)
    key_ = key_.reshape(B, S, H, DK).transpose(0, 2, 1, 3)
    value = value.reshape(B, S, H, DK).transpose(0, 2, 1, 3)
    scores = np.einsum("bhqd,bhkd->bhqk", query, key_) / np.sqrt(np.float32(DK))
    scores = np.where(mask == 0, np.float32(-1e9), scores)
    scores = scores - scores.max(axis=-1, keepdims=True)
    e = np.exp(scores)
    attn = e / e.sum(axis=-1, keepdims=True)
    x = np.einsum("bhqk,bhkd->bhqd", attn, value)
    x = x.transpose(0, 2, 1, 3).reshape(B, S, D)
    return (x @ wo.T + bo).astype(np.float32)


def kernel(q, k, v, mask, wq, bq, wk, bk, wv, bv, wo, bo, **_unused):
    q = np.asarray(q, np.float32)
    k = np.asarray(k, np.float32)
    v = np.asarray(v, np.float32)
    wq = np.asarray(wq, np.float32)
    wk = np.asarray(wk, np.float32)
    wv = np.asarray(wv, np.float32)
    wo = np.asarray(wo, np.float32)
    bq = np.asarray(bq, np.float32)
    bk = np.asarray(bk, np.float32)
    bv = np.asarray(bv, np.float32)
    bo = np.asarray(bo, np.float32)
    mask_np = np.asarray(mask)

    # the device kernel hardcodes causal masking; verify and fall back if not
    causal = np.tril(np.ones((S, S), np.int32))
    if not np.array_equal(mask_np.reshape(S, S).astype(np.int32), causal):
        return _reference_np(q, k, v, mask_np, wq, bq, wk, bk, wv, bv, wo, bo)

    with_bias = bool(np.any(bq) or np.any(bk) or np.any(bv))
    nc = _get_nc(with_bias)

    # host-side prep: transpose + cast to bf16 once per batch / core
    xT = {}
    for b in range(B):
        xT[b] = (q[b].T.astype(NPBF), k[b].T.astype(NPBF),
                 v[b].T.astype(NPBF))
    tri_np = np.triu(np.ones((P, P), NPBF))
    ones_np = np.ones((1, CH), NPBF)

    in_maps = []
    for c in range(NCORES):
        b, g = c // 4, c % 4
        sl = slice(g * HD, (g + 1) * HD)
        xq_b, xk_b, xv_b = xT[b]
        im = {
            "xqT": xq_b,
            "xkT": xk_b,
            "xvT": xv_b,
            "wqT": wq[sl].T.astype(NPBF),
            "wkT": wk[sl].T.astype(NPBF),
            "wvT": wv[sl].T.astype(NPBF),
            "woT": wo[:, sl].T.astype(NPBF),
            "tri": tri_np,
            "ones": ones_np,
        }
        if with_bias:
            im["bqc"] = np.ascontiguousarray(
                bq[sl].reshape(2, P).T.astype(np.float32))
            im["bkc"] = np.ascontiguousarray(
                bk[sl].reshape(2, P).T.astype(np.float32))
            im["bv"] = bv[sl].reshape(1, HD).astype(NPBF)
        in_maps.append(im)

    global _last_in_maps
    _last_in_maps = in_maps
    res = run_bass_kernel_spmd(nc, in_maps, core_ids=list(range(NCORES)))

    out = np.empty((B, S, D), np.float32)
    for b in range(B):
        acc = res.results[4 * b]["out"].astype(np.float32).copy()
        for g in range(1, 4):
            acc += res.results[4 * b + g]["out"]
        out[b] = acc + bo[None, :]
    return out


# revision 30
# speedup vs baseline: 1.0774x; 1.0298x over previous
"""Trainium2 Bass kernel for nn_MultiHeadAttention_79224966742350.

Full (unsharded) inputs in, full output out. Internally: 8-way SPMD over
8 NeuronCores, sharded batch x head-group: core c handles batch c//4 and
heads [4*(c%4), 4*(c%4)+4) (=256 of the 1024 projection dims). Each core
computes its partial x @ wo_cols contribution; the host sums the 4
partials per batch and adds bo.

v2 design notes (evolution of v1; v0 baseline was 360us, v1 194us):
  * All matmul operands bf16 (fp32 PSUM accumulation); inputs are
    pre-transposed AND pre-cast on the host, so the device does zero
    layout preparation (no PE transposes, no transpose evictions).
  * Scores for the two heads of a 128-row pair go into one [128,2,512]
    PSUM tile; a single ACT Exp instruction covers both heads.
  * Projection (chunk j+1) and output-projection (chunk j-1) matmuls
    are woven between attention iterations of chunk j so the PE never
    idles on the score->exp->attnV dependency chain.
  * Prologue: weight/x DMAs ordered so the Q-projection's first
    matmul can start after ~2us (wq first, then xq chunk 0 split per
    contraction tile; subtile deps release each matmul individually).
  * Bias matmuls are compiled out when all biases are zero (the case
    here); otherwise Q/K biases ride the PSUM eviction (ACT Identity
    with a per-partition bias AP - same ACT table as Exp, no reload),
    and V keeps a K=1 ones matmul.
  * Output DMAs issue from GPSIMD (25ns sequencer cost vs 565ns on
    sync) to keep the sync queue free for input prefetch.
"""

import sys

sys.path.insert(0, "/opt/trn_rl_repo")

import numpy as np
import ml_dtypes

import concourse.bacc as bacc
import concourse.mybir as mybir
import concourse.tile as tile
from concourse.bass_utils import run_bass_kernel_spmd

F32 = mybir.dt.float32
BF16 = mybir.dt.bfloat16
AF = mybir.ActivationFunctionType
NPBF = ml_dtypes.bfloat16

B = 2
S = 2048
D = 1024
DK = 64
HPC = 4          # heads per core
HD = HPC * DK    # 256 projection dims per core
NCORES = 8
CH = 512         # q-chunk width (tokens)
NJ = S // CH     # 4 chunks
P = 128


def build_nc(with_bias):
    nc = bacc.Bacc("TRN2", target_bir_lowering=False, debug=False,
                   num_devices=NCORES)

    xqT = nc.dram_tensor("xqT", [D, S], BF16, kind="ExternalInput").ap()
    xkT = nc.dram_tensor("xkT", [D, S], BF16, kind="ExternalInput").ap()
    xvT = nc.dram_tensor("xvT", [D, S], BF16, kind="ExternalInput").ap()
    wqT = nc.dram_tensor("wqT", [D, HD], BF16, kind="ExternalInput").ap()
    wkT = nc.dram_tensor("wkT", [D, HD], BF16, kind="ExternalInput").ap()
    wvT = nc.dram_tensor("wvT", [D, HD], BF16, kind="ExternalInput").ap()
    woT = nc.dram_tensor("woT", [HD, D], BF16, kind="ExternalInput").ap()
    if with_bias:
        # bqc/bkc: [128, 2] per-partition bias columns (pair-tiled dh)
        bqc = nc.dram_tensor("bqc", [P, 2], F32, kind="ExternalInput").ap()
        bkc = nc.dram_tensor("bkc", [P, 2], F32, kind="ExternalInput").ap()
        bv = nc.dram_tensor("bv", [1, HD], BF16, kind="ExternalInput").ap()
    tri = nc.dram_tensor("tri", [P, P], BF16, kind="ExternalInput").ap()
    ones = nc.dram_tensor("ones", [1, CH], BF16, kind="ExternalInput").ap()
    out = nc.dram_tensor("out", [S, D], F32, kind="ExternalOutput").ap()

    x_aps = {"q": xqT, "k": xkT, "v": xvT}

    with tile.TileContext(nc) as tc:
        with (
            tc.tile_pool(name="const", bufs=1) as const,
            tc.tile_pool(name="wtp", bufs=1) as wtp,
            tc.tile_pool(name="qkv", bufs=1) as qkv,
            tc.tile_pool(name="xtp", bufs=2) as xtp,
            tc.tile_pool(name="qc", bufs=2) as qcp,
            tc.tile_pool(name="opool", bufs=3) as opool,
            tc.tile_pool(name="pp", bufs=3) as pp,
            tc.tile_pool(name="small", bufs=2) as small,
            tc.tile_pool(name="outsb", bufs=3) as outsbp,
            tc.tile_pool(name="psS", bufs=2, space="PSUM") as psS,
            tc.tile_pool(name="psO", bufs=2, space="PSUM") as psO,
            tc.tile_pool(name="psPW", bufs=2, space="PSUM") as psPW,
        ):
            # ---- weight/const tiles ----------------------------------
            wq_sb = wtp.tile([P, 8, HD], BF16, tag="wq")
            wk_sb = wtp.tile([P, 8, HD], BF16, tag="wk")
            wv_sb = wtp.tile([P, 8, HD], BF16, tag="wv")
            wo_sb = wtp.tile([P, 2, D], BF16, tag="wo")
            tri_sb = const.tile([P, P], BF16, tag="tri")
            ones_sb = const.tile([1, CH], BF16, tag="ones")
            if with_bias:
                bqc_sb = wtp.tile([P, 2], F32, tag="bqc")
                bkc_sb = wtp.tile([P, 2], F32, tag="bkc")
                bv_sb = wtp.tile([1, HD], BF16, tag="bv")

            xts = {}       # (name, j) -> staged x^T tile
            q_tiles = {}   # j -> [Qc0, Qc1]
            o_tiles = {}   # j -> [O0, O1]

            def dma_x(j, split):
                """Stage x^T chunk j. split=True: one DMA per 128-row
                contraction tile (releases matmuls early via subtile
                deps); else one DMA per input."""
                for name in ("q", "k", "v"):
                    ap = x_aps[name].rearrange("(d p) t -> p d t", p=P)
                    xt = xtp.tile([P, 8, CH], BF16, tag=f"x{name}",
                                  name=f"x{name}{j}")
                    xts[(name, j)] = xt
                    if split:
                        for d in range(8):
                            nc.sync.dma_start(
                                out=xt[:, d, :],
                                in_=ap[:, d, j * CH:(j + 1) * CH])
                    else:
                        nc.sync.dma_start(
                            out=xt[:], in_=ap[:, :, j * CH:(j + 1) * CH])

            def prologue_dma():
                # ordered so the first Q-proj matmuls unblock earliest
                for name, wt, wsb in (("q", wqT, wq_sb), ("k", wkT, wk_sb),
                                      ("v", wvT, wv_sb)):
                    wap = wt.rearrange("(d p) h -> p d h", p=P)
                    xap = x_aps[name].rearrange("(d p) t -> p d t", p=P)
                    xt = xtp.tile([P, 8, CH], BF16, tag=f"x{name}",
                                  name=f"x{name}0")
                    xts[(name, 0)] = xt
                    nc.sync.dma_start(out=wsb[:], in_=wap)
                    for d in range(8):
                        nc.sync.dma_start(out=xt[:, d, :],
                                          in_=xap[:, d, 0:CH])
                nc.sync.dma_start(out=wo_sb[:],
                                  in_=woT.rearrange("(m p) n -> p m n", p=P))
                nc.sync.dma_start(out=tri_sb[:], in_=tri)
                nc.sync.dma_start(out=ones_sb[:], in_=ones)
                if with_bias:
                    nc.sync.dma_start(out=bqc_sb[:], in_=bqc)
                    nc.sync.dma_start(out=bkc_sb[:], in_=bkc)
                    nc.sync.dma_start(out=bv_sb[:], in_=bv)

            def proj_steps(j):
                """Generator: one PE matmul or eviction per yield."""
                q_tiles[j] = [qcp.tile([P, CH], BF16, tag=f"q{m}",
                                       name=f"Qc{m}_{j}")
                              for m in range(2)]
                for name, wsb in (("q", wq_sb), ("k", wk_sb)):
                    xt = xts[(name, j)]
                    for m in range(2):
                        ps = psPW.tile([P, CH], F32, tag="pw", name="psp")
                        for d in range(8):
                            nc.tensor.matmul(
                                ps[:], wsb[:, d, m * P:(m + 1) * P],
                                xt[:, d, :], start=(d == 0), stop=(d == 7))
                            yield
                        dst = (q_tiles[j][m][:] if name == "q"
                               else K_sb[m][:, j * CH:(j + 1) * CH])
                        if with_bias:
                            # DVE mult+add would cost more; ACT Identity
                            # shares the Exp table so no table reload.
                            bc = bqc_sb if name == "q" else bkc_sb
                            nc.scalar.activation(dst, ps[:], AF.Identity,
                                                 bias=bc[:, m:m + 1])
                        else:
                            # keep ACT exp-only: evict on DVE
                            nc.vector.tensor_copy(dst, ps[:])
                        yield
                xt = xts[("v", j)]
                for t in range(4):
                    ps = psPW.tile([P, CH], F32, tag="pw", name="psv")
                    for d in range(8):
                        nc.tensor.matmul(
                            ps[:, 0:HD], xt[:, d, t * P:(t + 1) * P],
                            wv_sb[:, d, :], start=(d == 0),
                            stop=(d == 7 and not with_bias))
                        yield
                    if with_bias:
                        nc.tensor.matmul(
                            ps[:, 0:HD], ones_sb[0:1, 0:P], bv_sb[:],
                            start=False, stop=True)
                        yield
                    nc.vector.tensor_copy(
                        V_sb[:, j * 4 + t, :, 0:DK],
                        ps[:, 0:HD].rearrange("p (h c) -> p h c", c=DK))
                    yield

            def wo_steps(j, act_ok=False):
                """Generator: output projection for chunk j. act_ok: the
                ACT engine has no more exps at this point, so osb
                evictions may alternate onto it."""
                O0, O1 = o_tiles[j]
                for t in range(4):
                    for n in range(2):
                        ps = psPW.tile([P, CH], F32, tag="pw", name="psw")
                        nc.tensor.matmul(
                            ps[:], O0[:, t * P:(t + 1) * P],
                            wo_sb[:, 0, n * CH:(n + 1) * CH],
                            start=True, stop=False)
                        yield
                        nc.tensor.matmul(
                            ps[:], O1[:, t * P:(t + 1) * P],
                            wo_sb[:, 1, n * CH:(n + 1) * CH],
                            start=False, stop=True)
                        yield
                        osb = outsbp.tile([P, CH], F32, tag="osb")
                        if act_ok and (t + n) % 2 == 0:
                            nc.scalar.activation(osb[:], ps[:], AF.Copy)
                        else:
                            nc.vector.tensor_copy(osb[:], ps[:])
                        yield
                        nc.sync.dma_start(
                            out=out[j * CH + t * P:j * CH + (t + 1) * P,
                                    n * CH:(n + 1) * CH],
                            in_=osb[:])
                        yield

            def do_att(j, weave_steps, total):
                """Attention for q-chunk j, weaving the given generators
                (`total` steps across them) into the PE stream between
                iterations."""
                weave = [iter(g) for g in weave_steps]

                def pump(n):
                    for _ in range(n):
                        while weave:
                            try:
                                next(weave[0])
                                break
                            except StopIteration:
                                weave.pop(0)
                        if not weave:
                            return

                o_tiles[j] = [opool.tile([P, CH], BF16, tag=f"o{m}",
                                         name=f"O{m}_{j}")
                              for m in range(2)]
                nk = 4 * (j + 1)
                # pace the weave to last the whole i-loop: running dry
                # leaves the tail iterations with no PE filler
                quota = max(1, total // (2 * nk))

                with nc.named_scope(f"att{j}"):
                    for pair in range(2):
                        m = pair
                        hA, hB = 2 * pair, 2 * pair + 1
                        Qc = q_tiles[j][m]
                        poA = psO.tile([DK + 1, CH], F32, tag="po",
                                       name="poA")
                        poB = psO.tile([DK + 1, CH], F32, tag="po",
                                       name="poB")

                        def c0(i):
                            return max(0, P * i - CH * j)

                        def score(i):
                            cc = c0(i)
                            s = psS.tile([P, 2, CH], F32, tag="s", name="s")
                            p = pp.tile([P, 2, CH], BF16, tag="p", name="p")
                            nc.tensor.matmul(
                                s[:, 0, cc:CH],
                                K_sb[m][0:DK, i * P:(i + 1) * P],
                                Qc[0:DK, cc:CH], start=True, stop=True)
                            nc.tensor.matmul(
                                s[:, 1, cc:CH],
                                K_sb[m][DK:P, i * P:(i + 1) * P],
                                Qc[DK:P, cc:CH], start=True, stop=True)
                            nc.scalar.activation(
                                p[:, :, cc:CH], s[:, :, cc:CH], AF.Exp,
                                scale=0.125)
                            if i >= 4 * j:
                                nc.vector.tensor_mul(
                                    p[:, 0, cc:cc + P], p[:, 0, cc:cc + P],
                                    tri_sb[:])
                                nc.vector.tensor_mul(
                                    p[:, 1, cc:cc + P], p[:, 1, cc:cc + P],
                                    tri_sb[:])
                            return p

                        prev = score(0)
                        for i in range(nk):
                            p = prev
                            if i + 1 < nk:
                                prev = score(i + 1)
                            cc = c0(i)
                            nc.tensor.matmul(
                                poA[:, cc:CH], V_sb[:, i, hA, :],
                                p[:, 0, cc:CH], start=(i == 0),
                                stop=(i == nk - 1))
                            nc.tensor.matmul(
                                poB[:, cc:CH], V_sb[:, i, hB, :],
                                p[:, 1, cc:CH], start=(i == 0),
                                stop=(i == nk - 1))
                            pump(quota)

                        for off, po in ((0, poA), (DK, poB)):
                            dsb = small.tile([1, CH], F32, tag="dsb")
                            nc.vector.tensor_copy(dsb[:], po[DK:DK + 1, :])
                            r = small.tile([1, CH], F32, tag="r")
                            nc.vector.reciprocal_approx_fast(r[:], dsb[:])
                            rb = small.tile([DK, CH], F32, tag="rb")
                            nc.gpsimd.partition_broadcast(rb[:], r[:],
                                                          channels=DK)
                            nc.vector.tensor_mul(
                                o_tiles[j][m][off:off + DK, :],
                                po[0:DK, :], rb[:])
                    pump(1 << 30)  # drain remaining weave steps

            # ---- persistent K^T (pair-tiled) and V (+ ones col) -------
            K_sb = [qkv.tile([P, S], BF16, tag=f"K{m}", name=f"K{m}")
                    for m in range(2)]
            V_sb = qkv.tile([P, S // P, HPC, DK + 1], BF16, tag="V")

            # ---- schedule ---------------------------------------------
            prologue_dma()
            nc.gpsimd.memset(V_sb[:, :, :, DK:DK + 1], 1.0)
            dma_x(1, split=False)
            with nc.named_scope("proj0"):
                for _ in proj_steps(0):
                    pass
            dma_x(2, split=False)
            do_att(0, [proj_steps(1)], total=72)
            dma_x(3, split=False)
            do_att(1, [wo_steps(0), proj_steps(2)], total=104)
            do_att(2, [proj_steps(3)], total=72)
            do_att(3, [wo_steps(1), wo_steps(2)], total=64)
            with nc.named_scope("wo3"):
                for _ in wo_steps(3, act_ok=True):
                    pass

    nc.compile()
    return nc


_NC_CACHE = {}
_last_in_maps = None


def _get_nc(with_bias=False):
    if with_bias not in _NC_CACHE:
        _NC_CACHE[with_bias] = build_nc(with_bias)
    return _NC_CACHE[with_bias]


def _reference_np(q, k, v, mask, wq, bq, wk, bk, wv, bv, wo, bo):
    """Plain numpy fallback (only used if mask is not causal)."""
    query = q @ wq.T + bq
    key_ = k @ wk.T + bk
    value = v @ wv.T + bv
    H = D // DK
    query = query.reshape(B, S, H, DK).transpose(0, 2, 1, 3)
    key_ = key_.reshape(B, S, H, DK).transpose(0, 2, 1, 3)
    value = value.reshape(B, S, H, DK).transpose(0, 2, 1, 3)
    scores = np.einsum("bhqd,bhkd->bhqk", query, key_) / np.sqrt(np.float32(DK))
    scores = np.where(mask == 0, np.float32(-1e9), scores)
    scores = scores - scores.max(axis=-1, keepdims=True)
    e = np.exp(scores)
    attn = e / e.sum(axis=-1, keepdims=True)
    x = np.einsum("bhqk,bhkd->bhqd", attn, value)
    x = x.transpose(0, 2, 1, 3).reshape(B, S, D)
    return (x @ wo.T + bo).astype(np.float32)


def kernel(q, k, v, mask, wq, bq, wk, bk, wv, bv, wo, bo, **_unused):
    q = np.asarray(q, np.float32)
    k = np.asarray(k, np.float32)
    v = np.asarray(v, np.float32)
    wq = np.asarray(wq, np.float32)
    wk = np.asarray(wk, np.float32)
    wv = np.asarray(wv, np.float32)
    wo = np.asarray(wo, np.float32)
    bq = np.asarray(bq, np.float32)
    bk = np.asarray(bk, np.float32)
    bv = np.asarray(bv, np.float32)
    bo = np.asarray(bo, np.float32)
    mask_np = np.asarray(mask)

    # the device kernel hardcodes causal masking; verify and fall back if not
    causal = np.tril(np.ones((S, S), np.int32))
    if not np.array_equal(mask_np.reshape(S, S).astype(np.int32), causal):
        return _reference_np(q, k, v, mask_np, wq, bq, wk, bk, wv, bv, wo, bo)

    with_bias = bool(np.any(bq) or np.any(bk) or np.any(bv))
    nc = _get_nc(with_bias)

    # host-side prep: transpose + cast to bf16 once per batch / core
    xT = {}
    for b in range(B):
        xT[b] = (q[b].T.astype(NPBF), k[b].T.astype(NPBF),
                 v[b].T.astype(NPBF))
    tri_np = np.triu(np.ones((P, P), NPBF))
    ones_np = np.ones((1, CH), NPBF)

    in_maps = []
    for c in range(NCORES):
        b, g = c // 4, c % 4
        sl = slice(g * HD, (g + 1) * HD)
        xq_b, xk_b, xv_b = xT[b]
        im = {
            "xqT": xq_b,
            "xkT": xk_b,
            "xvT": xv_b,
            "wqT": wq[sl].T.astype(NPBF),
            "wkT": wk[sl].T.astype(NPBF),
            "wvT": wv[sl].T.astype(NPBF),
            "woT": wo[:, sl].T.astype(NPBF),
            "tri": tri_np,
            "ones": ones_np,
        }
        if with_bias:
            im["bqc"] = np.ascontiguousarray(
                bq[sl].reshape(2, P).T.astype(np.float32))
            im["bkc"] = np.ascontiguousarray(
                bk[sl].reshape(2, P).T.astype(np.float32))
            im["bv"] = bv[sl].reshape(1, HD).astype(NPBF)
        in_maps.append(im)

    global _last_in_maps
    _last_in_maps = in_maps
    res = run_bass_kernel_spmd(nc, in_maps, core_ids=list(range(NCORES)))

    out = np.empty((B, S, D), np.float32)
    for b in range(B):
        acc = res.results[4 * b]["out"].astype(np.float32).copy()
        for g in range(1, 4):
            acc += res.results[4 * b + g]["out"]
        out[b] = acc + bo[None, :]
    return out
